# revision 1
# baseline (speedup 1.0000x reference)
"""Trainium2 Bass kernel for the ChipletThermalModel problem.

Math (per batch row, per grid point g, summed over 16 chiplets i):
  u = (x - cx_i)/lx_i ; v = (y - cy_i)/ly_i
  b± = w_i/(2 lx_i) ± u ; c± = h_i/(2 ly_i) ± v
  For each of the 4 (b,c) sign combos:
    S = a² + b² + c² ; δ = √S ;  (δ via exp(½·lnS), 1/δ via exp(-½·lnS))
    t1 = b·(ln(c+δ) - ½ln(a²+b²)) ; t2 = c·(ln(b+δ) - ½ln(a²+c²))
    t3 = a·atan(b·c/(a·δ))
  result += P_i·A·(B_off + 2/√π · Σ(t1+t2-t3))

Sharding: batch dim (64) split across 8 cores -> 8 rows/core, laid out as
[128 partitions, 4096] (each batch row spans 16 partitions). Per-batch-row
chiplet parameters become per-partition [128,1] scalars (host-precomputed);
per-chiplet scalars (1/lx_i, a², ...) are baked as immediates.

Engine split per chiplet: ACT does squares + all transcendentals (grouped by
activation-table set: any -> ln/exp -> trig, 2 table switches per chiplet),
Pool (gpsimd) does half the tensor_tensor adds, DVE the rest.
"""
import sys
import numpy as np

for _p in ("/opt/trn_rl_repo",):
    if _p not in sys.path:
        sys.path.insert(0, _p)

N_CORES = 8
B, NCHIP, G2 = 64, 16, 65536
RPC = B // N_CORES            # batch rows per core = 8
P = 128                       # SBUF partitions
F = RPC * G2 // P             # free-dim columns per core = 4096
W = 1024                      # columns per processing group
NG = F // W                   # groups
REP = P // RPC                # partitions per batch row = 16
NPAR = 6 * NCHIP + 1          # params columns (6 per chiplet + endC)
C1 = float(2.0 / np.sqrt(np.pi))


def _build_program(scal):
    """Build the Bass program. `scal` holds python-float per-chiplet scalars."""
    from concourse import bacc, tile
    import concourse.mybir as mybir

    AF = mybir.ActivationFunctionType
    OP = mybir.AluOpType
    FP32 = mybir.dt.float32

    nc = bacc.Bacc("TRN2", target_bir_lowering=False, debug=False,
                   enable_asserts=False)

    xin = nc.dram_tensor("xin", [P, F], FP32, kind="ExternalInput")
    yin = nc.dram_tensor("yin", [P, F], FP32, kind="ExternalInput")
    prm = nc.dram_tensor("prm", [P, NPAR], FP32, kind="ExternalInput")
    out = nc.dram_tensor("out", [P, F], FP32, kind="ExternalOutput")

    a2 = scal["a2"]
    inv_a = scal["inv_a"]
    neg_a = scal["neg_a"]
    inv_lx = scal["inv_lx"]
    inv_ly = scal["inv_ly"]

    with tile.TileContext(nc) as tc:
        with tc.tile_pool(name="cst", bufs=1) as cst, \
             tc.tile_pool(name="io", bufs=2) as io, \
             tc.tile_pool(name="wk", bufs=38) as wk:
            prmt = cst.tile([P, NPAR], FP32)
            nc.sync.dma_start(prmt[:], prm[:])

            def pcol(i, k):           # [128,1] per-partition param AP
                return prmt[:, 6 * i + k: 6 * i + k + 1]

            endC = prmt[:, 6 * NCHIP: 6 * NCHIP + 1]

            for g in range(NG):
                cs = slice(g * W, (g + 1) * W)
                xt = io.tile([P, W], FP32, tag="xt")
                yt = io.tile([P, W], FP32, tag="yt")
                res = io.tile([P, W], FP32, tag="res")
                nc.sync.dma_start(xt[:], xin[:, cs])
                nc.sync.dma_start(yt[:], yin[:, cs])

                def wtile(nm):
                    return wk.tile([P, W], FP32, tag="wk", name=nm)

                for i in range(NCHIP):
                    # ---- affine prologue (DVE tensor_scalar, 2x mode) ----
                    u = wtile("u"); v = wtile("v")
                    nc.vector.tensor_scalar(u[:], xt[:], inv_lx[i], pcol(i, 0),
                                            OP.mult, OP.add)
                    nc.vector.tensor_scalar(v[:], yt[:], inv_ly[i], pcol(i, 2),
                                            OP.mult, OP.add)
                    bm = wtile("bm"); bp = wtile("bp")
                    cm = wtile("cm"); cp = wtile("cp")
                    nc.vector.tensor_scalar(bm[:], u[:], -1.0, pcol(i, 1),
                                            OP.mult, OP.add)
                    nc.vector.tensor_scalar(bp[:], u[:], pcol(i, 1), None,
                                            OP.add)
                    nc.vector.tensor_scalar(cm[:], v[:], -1.0, pcol(i, 3),
                                            OP.mult, OP.add)
                    nc.vector.tensor_scalar(cp[:], v[:], pcol(i, 3), None,
                                            OP.add)
                    bs = {"m": bm, "p": bp}
                    cs_ = {"m": cm, "p": cp}
                    # squares (Square is in every ACT table set - no switch)
                    sqb = {}; sqc = {}
                    for k in "mp":
                        sb = wtile("sqb"); sc = wtile("sqc")
                        nc.scalar.activation(sb[:], bs[k][:], AF.Square)
                        nc.scalar.activation(sc[:], cs_[k][:], AF.Square)
                        sqb[k] = sb; sqc[k] = sc
                    # s0 = b² + c² per combo (Pool)
                    s0s = {}
                    for kx in "mp":
                        for ky in "mp":
                            s0 = wtile("s0")
                            nc.gpsimd.tensor_tensor(
                                s0[:], sqb[kx][:], sqc[ky][:], OP.add)
                            s0s[kx + ky] = s0
                    # ---- ln/exp table set ----
                    laxb = {}; laxc = {}
                    for k in "mp":
                        lb = wtile("laxb"); lc = wtile("laxc")
                        nc.scalar.activation(lb[:], sqb[k][:], AF.Ln, bias=a2)
                        nc.scalar.activation(lc[:], sqc[k][:], AF.Ln, bias=a2)
                        laxb[k] = lb; laxc[k] = lc
                    dls = {}; rds = {}
                    for kk, s0 in s0s.items():
                        lS = wtile("lS")
                        nc.scalar.activation(lS[:], s0[:], AF.Ln, bias=a2)
                        dl = wtile("dl"); rd = wtile("rd")
                        nc.scalar.activation(dl[:], lS[:], AF.Exp, scale=0.5)
                        nc.scalar.activation(rd[:], lS[:], AF.Exp, scale=-0.5)
                        dls[kk] = dl; rds[kk] = rd
                    # per-combo: c+δ, b+δ (Pool), their lns (ACT), atan arg (DVE)
                    qs = []
                    lbds = {}
                    targs = {}
                    for kx in "mp":
                        lcds = {}
                        for ky in "mp":
                            kk = kx + ky
                            cpd = wtile("cpd"); bpd = wtile("bpd")
                            nc.gpsimd.tensor_tensor(
                                cpd[:], cs_[ky][:], dls[kk][:], OP.add)
                            nc.gpsimd.tensor_tensor(
                                bpd[:], bs[kx][:], dls[kk][:], OP.add)
                            lcd = wtile("lcd"); lbd = wtile("lbd")
                            nc.scalar.activation(lcd[:], cpd[:], AF.Ln)
                            nc.scalar.activation(lbd[:], bpd[:], AF.Ln)
                            lcds[ky] = lcd; lbds[kk] = lbd
                            bc = wtile("bc")
                            nc.vector.tensor_tensor(
                                bc[:], bs[kx][:], cs_[ky][:], OP.mult)
                            targ = wtile("targ")
                            nc.vector.tensor_tensor(
                                targ[:], bc[:], rds[kk][:], OP.mult)
                            targs[kk] = targ
                        # b-side partial: q = b_x·(ln(c+δ)_xm + ln(c+δ)_xp - lax_bx)
                        pm = wtile("pm")
                        nc.gpsimd.tensor_tensor(pm[:], lcds["m"][:],
                                                lcds["p"][:], OP.add)
                        Lb = wtile("Lb")
                        nc.vector.tensor_tensor(Lb[:], pm[:], laxb[kx][:],
                                                OP.subtract)
                        q = wtile("q")
                        nc.vector.tensor_tensor(q[:], bs[kx][:], Lb[:], OP.mult)
                        qs.append(q)
                    for ky in "mp":   # c-side partials
                        pm = wtile("pm")
                        nc.gpsimd.tensor_tensor(
                            pm[:], lbds["m" + ky][:], lbds["p" + ky][:], OP.add)
                        Lc = wtile("Lb")
                        nc.vector.tensor_tensor(Lc[:], pm[:], laxc[ky][:],
                                                OP.subtract)
                        q = wtile("q")
                        nc.vector.tensor_tensor(q[:], cs_[ky][:], Lc[:],
                                                OP.mult)
                        qs.append(q)
                    # ---- trig table set ----
                    ats = {}
                    for kk, targ in targs.items():
                        at = wtile("at")
                        nc.scalar.activation(at[:], targ[:], AF.Arctan,
                                             scale=inv_a)
                        ats[kk] = at
                    # ---- combine (DVE) ----
                    z1 = wtile("z1"); z2 = wtile("z2"); z12 = wtile("z12")
                    nc.vector.tensor_tensor(z1[:], qs[0][:], qs[1][:], OP.add)
                    nc.vector.tensor_tensor(z2[:], qs[2][:], qs[3][:], OP.add)
                    nc.vector.tensor_tensor(z12[:], z1[:], z2[:], OP.add)
                    A12 = wtile("A12"); A34 = wtile("A34"); At = wtile("At")
                    nc.vector.tensor_tensor(A12[:], ats["mm"][:], ats["mp"][:],
                                            OP.add)
                    nc.vector.tensor_tensor(A34[:], ats["pm"][:], ats["pp"][:],
                                            OP.add)
                    nc.vector.tensor_tensor(At[:], A12[:], A34[:], OP.add)
                    zz = wtile("zz")
                    nc.vector.scalar_tensor_tensor(zz[:], At[:], neg_a, z12[:],
                                                   OP.mult, OP.add)
                    if i == 0:
                        # res = zz·(P_i·A·2/√π) + endC  (endC = A·B_off·ΣP_i)
                        nc.vector.tensor_scalar(res[:], zz[:], pcol(i, 4),
                                                endC, OP.mult, OP.add)
                    else:
                        nc.vector.scalar_tensor_tensor(res[:], zz[:],
                                                       pcol(i, 4), res[:],
                                                       OP.mult, OP.add)
                nc.sync.dma_start(out[:, cs], res[:])
    nc.finalize()
    return nc


def _host_params(cx, cy, w, h, Pw, A, a, B_off, lx, ly, rows):
    """Per-core [128, NPAR] parameter matrix (per-partition scalars)."""
    pr = np.zeros((P, NPAR), dtype=np.float32)
    for i in range(NCHIP):
        pr[:, 6 * i + 0] = np.repeat(-cx[rows, i] / lx[i], REP)
        pr[:, 6 * i + 1] = np.repeat(0.5 * w[rows, i] / lx[i], REP)
        pr[:, 6 * i + 2] = np.repeat(-cy[rows, i] / ly[i], REP)
        pr[:, 6 * i + 3] = np.repeat(0.5 * h[rows, i] / ly[i], REP)
        pr[:, 6 * i + 4] = np.repeat(Pw[rows, i] * A * C1, REP)
    pr[:, 6 * NCHIP] = np.repeat(A * B_off * Pw[rows].sum(axis=1), REP)
    return np.ascontiguousarray(pr, dtype=np.float32)


_CACHE = {}


def run(x, y, chiplets_x, chiplets_y, chiplets_width, chiplets_height,
        chiplets_power, A, a, B_off, lx, ly, grid=None, trace=False):
    from concourse import bass_utils

    x = np.asarray(x, dtype=np.float32)
    y = np.asarray(y, dtype=np.float32)
    cx = np.asarray(chiplets_x, dtype=np.float32)
    cy = np.asarray(chiplets_y, dtype=np.float32)
    w = np.asarray(chiplets_width, dtype=np.float32)
    h = np.asarray(chiplets_height, dtype=np.float32)
    Pw = np.asarray(chiplets_power, dtype=np.float32)
    Af = float(np.asarray(A).reshape(-1)[0])
    af = float(np.asarray(a).reshape(-1)[0])
    Bf = float(np.asarray(B_off).reshape(-1)[0])
    lxf = np.asarray(lx, dtype=np.float64)
    lyf = np.asarray(ly, dtype=np.float64)

    scal = {
        "a2": float(af * af),
        "inv_a": float(1.0 / af),
        "neg_a": float(-af),
        "inv_lx": [float(1.0 / lxf[i]) for i in range(NCHIP)],
        "inv_ly": [float(1.0 / lyf[i]) for i in range(NCHIP)],
    }
    if "nc" not in _CACHE:
        _CACHE["nc"] = _build_program(scal)
    nc = _CACHE["nc"]

    in_maps = []
    for c in range(N_CORES):
        rows = slice(c * RPC, (c + 1) * RPC)
        xs = np.ascontiguousarray(x[rows].reshape(P, F))
        ys = np.ascontiguousarray(y[rows].reshape(P, F))
        pr = _host_params(cx, cy, w, h, Pw, Af, af, Bf, lxf, lyf, rows)
        in_maps.append({"xin": xs, "yin": ys, "prm": pr})

    rr = bass_utils.run_bass_kernel_spmd(
        nc, in_maps, core_ids=list(range(N_CORES)), trace=trace)

    outs = []
    for c in range(N_CORES):
        o = np.asarray(rr.results[c]["out"], dtype=np.float32)
        outs.append(o.reshape(RPC, G2))
    full = np.concatenate(outs, axis=0)
    if trace:
        return full, rr
    return full


def kernel(**inputs):
    return run(**inputs)



# revision 4
# speedup vs baseline: 1.2058x; 1.2058x over previous
"""Trainium2 Bass kernel for the ChipletThermalModel problem.

Math (per batch row, per grid point g, summed over 16 chiplets i):
  b± = w_i/(2 lx_i) ∓ (x - cx_i)/lx_i ; c± = h_i/(2 ly_i) ∓ (y - cy_i)/ly_i
  For each of the 4 (b,c) sign combos k:
    S_k = a² + b² + c² ; δ_k = √S_k  (via exp(½·ln S_k)), r_k = 1/δ_k
  Σ_k t1+t2 = Σ_b b·[ln((cm+δ)(cp+δ)) − ln(a²+b²)]
            + Σ_c c·[ln((bm+δ)(bp+δ)) − ln(a²+c²)]
  Σ_k t3 = a·Σ_k atan(b·c·r_k/a)
  result += P_i·A·(B_off + 2/√π·(Σt1+t2 − Σt3))

Sharding: batch dim (64) split across 8 cores -> 8 rows/core, laid out as
[128 partitions, 4096] (each batch row spans 16 partitions). Per-batch-row
chiplet parameters become per-partition [128,1] scalars (host-precomputed);
per-chiplet scalars (1/lx_i, a², ...) are baked as immediates.

W=2048 column tiles (2 groups) to halve instruction count vs W=1024; the
working set is packed into 22 [128,2048] tiles via in-place ops and manual
slot aliasing (address-based dependency tracking makes this safe).
Per chiplet: 24 DVE + 24 Pool + 24 ACT instructions, 2 ACT table switches
(ln/exp set -> trig set).
"""
import sys
import numpy as np

for _p in ("/opt/trn_rl_repo",):
    if _p not in sys.path:
        sys.path.insert(0, _p)

N_CORES = 8
B, NCHIP, G2 = 64, 16, 65536
RPC = B // N_CORES            # batch rows per core = 8
P = 128                       # SBUF partitions
F = RPC * G2 // P             # free-dim columns per core = 4096
REP = P // RPC                # partitions per batch row = 16
NPAR = 5 * NCHIP + 1          # params columns (5 per chiplet + endC)
C1 = float(2.0 / np.sqrt(np.pi))


def _build_program(scal, W):
    """Build the Bass program. `scal` holds python-float per-chiplet scalars."""
    from concourse import bacc, tile
    import concourse.mybir as mybir

    AF = mybir.ActivationFunctionType
    OP = mybir.AluOpType
    FP32 = mybir.dt.float32
    NG = F // W

    nc = bacc.Bacc("TRN2", target_bir_lowering=False, debug=False,
                   enable_asserts=False)

    xin = nc.dram_tensor("xin", [P, F], FP32, kind="ExternalInput")
    yin = nc.dram_tensor("yin", [P, F], FP32, kind="ExternalInput")
    prm = nc.dram_tensor("prm", [P, NPAR], FP32, kind="ExternalInput")
    out = nc.dram_tensor("out", [P, F], FP32, kind="ExternalOutput")

    a2 = scal["a2"]
    inv_a = scal["inv_a"]
    neg_a = scal["neg_a"]
    inv_lx = scal["inv_lx"]
    inv_ly = scal["inv_ly"]

    with tile.TileContext(nc) as tc:
        with tc.tile_pool(name="cst", bufs=1) as cst, \
             tc.tile_pool(name="io", bufs=1) as io, \
             tc.tile_pool(name="wk", bufs=1) as wk:
            prmt = cst.tile([P, NPAR], FP32, tag="prmt")
            nc.sync.dma_start(prmt[:], prm[:])
            a2c = cst.tile([P, 1], FP32, tag="a2c")
            nc.gpsimd.memset(a2c[:], a2)

            def pcol(i, k):           # [128,1] per-partition param AP
                return prmt[:, 5 * i + k: 5 * i + k + 1]

            endC = prmt[:, 5 * NCHIP: 5 * NCHIP + 1]

            # work tiles, allocated once (tag-per-tile => stable buffers),
            # reused every chiplet/group
            def wt(nm):
                return wk.tile([P, W], FP32, name=nm, tag=nm)

            BM, BP, CM, CP = wt("BM"), wt("BP"), wt("CM"), wt("CP")
            SQB = [wt(f"SQB{j}") for j in range(2)]
            S0 = [wt(f"S0{j}") for j in range(4)]
            LAX = [wt(f"LAX{j}") for j in range(4)]
            DL = [wt(f"DL{j}") for j in range(4)]
            R = [wt(f"R{j}") for j in range(4)]

            V, PL, A = nc.vector, nc.gpsimd, nc.scalar

            for g in range(NG):
                cs = slice(g * W, (g + 1) * W)
                xt = io.tile([P, W], FP32, tag="xt")
                yt = io.tile([P, W], FP32, tag="yt")
                res = io.tile([P, W], FP32, tag="res")
                nc.sync.dma_start(xt[:], xin[:, cs])
                nc.sync.dma_start(yt[:], yin[:, cs])

                for i in range(NCHIP):
                    # affine prologue: b± = ±x/lx + (w/2lx ∓ cx/lx)
                    V.tensor_scalar(BM[:], xt[:], -inv_lx[i], pcol(i, 0),
                                    OP.mult, OP.add)
                    V.tensor_scalar(BP[:], xt[:], inv_lx[i], pcol(i, 1),
                                    OP.mult, OP.add)
                    V.tensor_scalar(CM[:], yt[:], -inv_ly[i], pcol(i, 2),
                                    OP.mult, OP.add)
                    V.tensor_scalar(CP[:], yt[:], inv_ly[i], pcol(i, 3),
                                    OP.mult, OP.add)
                    # squares: sqb in SQB, sqc parked in S0[0]/S0[3]
                    PL.tensor_tensor(SQB[0][:], BM[:], BM[:], OP.mult)
                    PL.tensor_tensor(SQB[1][:], BP[:], BP[:], OP.mult)
                    V.tensor_tensor(S0[0][:], CM[:], CM[:], OP.mult)
                    V.tensor_tensor(S0[3][:], CP[:], CP[:], OP.mult)
                    # lax = ln(a² + sq)   [ln/exp table set]
                    A.activation(LAX[0][:], SQB[0][:], AF.Ln, bias=a2c[:])
                    A.activation(LAX[1][:], SQB[1][:], AF.Ln, bias=a2c[:])
                    A.activation(LAX[2][:], S0[0][:], AF.Ln, bias=a2c[:])
                    A.activation(LAX[3][:], S0[3][:], AF.Ln, bias=a2c[:])
                    # s0_k = sqb_kx + sqc_ky ; k = (mm, mp, pm, pp)
                    PL.tensor_tensor(S0[1][:], SQB[0][:], S0[3][:], OP.add)
                    PL.tensor_tensor(S0[2][:], SQB[1][:], S0[0][:], OP.add)
                    PL.tensor_tensor(S0[0][:], SQB[0][:], S0[0][:], OP.add)
                    PL.tensor_tensor(S0[3][:], SQB[1][:], S0[3][:], OP.add)
                    # lS = ln(a²+s0) in-place; δ = exp(½lS); r = exp(-½lS)
                    for k in range(4):
                        A.activation(S0[k][:], S0[k][:], AF.Ln, bias=a2c[:])
                    for k in range(4):
                        A.activation(DL[k][:], S0[k][:], AF.Exp, scale=0.5)
                    for k in range(4):
                        A.activation(R[k][:], S0[k][:], AF.Exp, scale=-0.5)
                    # bc_k = b_kx·c_ky into S0 slots; targ = bc·r in-place
                    bsel = (BM, BM, BP, BP)
                    csel = (CM, CP, CM, CP)
                    for k in range(4):
                        V.tensor_tensor(S0[k][:], bsel[k][:], csel[k][:],
                                        OP.mult)
                    for k in range(4):
                        V.tensor_tensor(S0[k][:], S0[k][:], R[k][:], OP.mult)
                    # bpd = b+δ into R slots; cpd = c+δ in-place in DL
                    for k in range(4):
                        PL.tensor_tensor(R[k][:], bsel[k][:], DL[k][:], OP.add)
                    for k in range(4):
                        PL.tensor_tensor(DL[k][:], csel[k][:], DL[k][:], OP.add)
                    # log-product merge
                    V.tensor_tensor(DL[0][:], DL[0][:], DL[1][:], OP.mult)
                    V.tensor_tensor(DL[2][:], DL[2][:], DL[3][:], OP.mult)
                    PL.tensor_tensor(R[0][:], R[0][:], R[2][:], OP.mult)
                    PL.tensor_tensor(R[1][:], R[1][:], R[3][:], OP.mult)
                    A.activation(DL[0][:], DL[0][:], AF.Ln)
                    A.activation(DL[2][:], DL[2][:], AF.Ln)
                    A.activation(R[0][:], R[0][:], AF.Ln)
                    A.activation(R[1][:], R[1][:], AF.Ln)
                    PL.tensor_tensor(DL[0][:], DL[0][:], LAX[0][:], OP.subtract)
                    PL.tensor_tensor(DL[2][:], DL[2][:], LAX[1][:], OP.subtract)
                    PL.tensor_tensor(R[0][:], R[0][:], LAX[2][:], OP.subtract)
                    PL.tensor_tensor(R[1][:], R[1][:], LAX[3][:], OP.subtract)
                    # q terms into LAX slots, tree-sum into LAX[0]
                    V.tensor_tensor(LAX[0][:], BM[:], DL[0][:], OP.mult)
                    V.tensor_tensor(LAX[1][:], BP[:], DL[2][:], OP.mult)
                    V.tensor_tensor(LAX[2][:], CM[:], R[0][:], OP.mult)
                    V.tensor_tensor(LAX[3][:], CP[:], R[1][:], OP.mult)
                    PL.tensor_tensor(LAX[0][:], LAX[0][:], LAX[1][:], OP.add)
                    PL.tensor_tensor(LAX[2][:], LAX[2][:], LAX[3][:], OP.add)
                    PL.tensor_tensor(LAX[0][:], LAX[0][:], LAX[2][:], OP.add)
                    # atans in-place on targ (S0 slots)   [trig table set]
                    for k in range(4):
                        A.activation(S0[k][:], S0[k][:], AF.Arctan,
                                     scale=inv_a)
                    V.tensor_tensor(S0[0][:], S0[0][:], S0[1][:], OP.add)
                    PL.tensor_tensor(S0[2][:], S0[2][:], S0[3][:], OP.add)
                    V.tensor_tensor(S0[0][:], S0[0][:], S0[2][:], OP.add)
                    # zz = -a·Atot + qsum ; res (+)= P'·zz
                    V.scalar_tensor_tensor(LAX[1][:], S0[0][:], neg_a,
                                           LAX[0][:], OP.mult, OP.add)
                    if i == 0:
                        V.tensor_scalar(res[:], LAX[1][:], pcol(i, 4), endC,
                                        OP.mult, OP.add)
                    else:
                        V.scalar_tensor_tensor(res[:], LAX[1][:], pcol(i, 4),
                                               res[:], OP.mult, OP.add)
                nc.sync.dma_start(out[:, cs], res[:])
    nc.finalize()
    return nc


def _host_params(cx, cy, w, h, Pw, A, a, B_off, lx, ly, rows):
    """Per-core [128, NPAR] parameter matrix (per-partition scalars)."""
    pr = np.zeros((P, NPAR), dtype=np.float32)
    for i in range(NCHIP):
        w2l = 0.5 * w[rows, i] / lx[i]
        cxl = cx[rows, i] / lx[i]
        h2l = 0.5 * h[rows, i] / ly[i]
        cyl = cy[rows, i] / ly[i]
        pr[:, 5 * i + 0] = np.repeat(w2l + cxl, REP)   # for BM
        pr[:, 5 * i + 1] = np.repeat(w2l - cxl, REP)   # for BP
        pr[:, 5 * i + 2] = np.repeat(h2l + cyl, REP)   # for CM
        pr[:, 5 * i + 3] = np.repeat(h2l - cyl, REP)   # for CP
        pr[:, 5 * i + 4] = np.repeat(Pw[rows, i] * A * C1, REP)
    pr[:, 5 * NCHIP] = np.repeat(A * B_off * Pw[rows].sum(axis=1), REP)
    return np.ascontiguousarray(pr, dtype=np.float32)


_CACHE = {}


def run(x, y, chiplets_x, chiplets_y, chiplets_width, chiplets_height,
        chiplets_power, A, a, B_off, lx, ly, grid=None, trace=False):
    from concourse import bass_utils

    x = np.asarray(x, dtype=np.float32)
    y = np.asarray(y, dtype=np.float32)
    cx = np.asarray(chiplets_x, dtype=np.float32)
    cy = np.asarray(chiplets_y, dtype=np.float32)
    w = np.asarray(chiplets_width, dtype=np.float32)
    h = np.asarray(chiplets_height, dtype=np.float32)
    Pw = np.asarray(chiplets_power, dtype=np.float32)
    Af = float(np.asarray(A).reshape(-1)[0])
    af = float(np.asarray(a).reshape(-1)[0])
    Bf = float(np.asarray(B_off).reshape(-1)[0])
    lxf = np.asarray(lx, dtype=np.float64)
    lyf = np.asarray(ly, dtype=np.float64)

    scal = {
        "a2": float(af * af),
        "inv_a": float(1.0 / af),
        "neg_a": float(-af),
        "inv_lx": [float(1.0 / lxf[i]) for i in range(NCHIP)],
        "inv_ly": [float(1.0 / lyf[i]) for i in range(NCHIP)],
    }
    if "nc" not in _CACHE:
        try:
            _CACHE["nc"] = _build_program(scal, W=2048)
        except Exception as e:
            print(f"kernel: W=2048 build failed ({e}); falling back to W=1024",
                  file=sys.stderr)
            _CACHE["nc"] = _build_program(scal, W=1024)
    nc = _CACHE["nc"]

    in_maps = []
    for c in range(N_CORES):
        rows = slice(c * RPC, (c + 1) * RPC)
        xs = np.ascontiguousarray(x[rows].reshape(P, F))
        ys = np.ascontiguousarray(y[rows].reshape(P, F))
        pr = _host_params(cx, cy, w, h, Pw, Af, af, Bf, lxf, lyf, rows)
        in_maps.append({"xin": xs, "yin": ys, "prm": pr})

    rr = bass_utils.run_bass_kernel_spmd(
        nc, in_maps, core_ids=list(range(N_CORES)), trace=trace)

    outs = []
    for c in range(N_CORES):
        o = np.asarray(rr.results[c]["out"], dtype=np.float32)
        outs.append(o.reshape(RPC, G2))
    full = np.concatenate(outs, axis=0)
    if trace:
        return full, rr
    return full


def kernel(**inputs):
    return run(**inputs)
